# revision 29
# baseline (speedup 1.0000x reference)
"""CalScaleOPTAttention on 8 TRN2 NeuronCores.

Sharding: heads across cores (2 heads / core, 256 channels each).
Device-side compute keeps all quantization arithmetic; quantized values are
small integers, exact in bf16, so projection matmuls run as int-bf16 with the
rank-1 scale factors applied after the matmul (more accurate than fp32).
Attention matmuls (unquantized q/k, softmax probs) use fp32r (TF32-like,
11-bit mantissa). Causal masking is exploited structurally: column blocks
beyond the diagonal are never computed (exp == 0 exactly).

Collectives: AllGather(act scales), AllReduce-max(k/v quant stats),
AllReduce-add(accumulated attention score), AllGather(ctx + ctx absmax).
"""

import numpy as np
import ml_dtypes

import concourse.bass as bass
import concourse.mybir as mybir
import concourse.tile as tile
from concourse.tile import add_dep_helper
from concourse import bacc
from concourse.bass_utils import run_bass_kernel_spmd

F32 = mybir.dt.float32
F32R = mybir.dt.float32r
BF16 = mybir.dt.bfloat16
AX = mybir.AxisListType
OP = mybir.AluOpType
ACTF = mybir.ActivationFunctionType

NCORES = 8
T = 2048
E = 2048
H = 16
D = 128                   # head dim
HL = H // NCORES          # heads per core = 2
CH = HL * D               # channels per core = 256
NT = T // 128             # 16 row tiles
NE = E // 128             # 16 contraction tiles
Q8 = 127.0
Q4 = 7.0
EPS = 1e-5
NEG = -1e9
RND_C = 12582912.0        # 1.5 * 2**23 round-to-int trick constant
SCALING = float(D) ** -0.5
K_TOP = T // 40           # 51
AGW = CH * T + T          # ctx allgather row width per core


def _cdiv(a, b):
    return (a + b - 1) // b


DEBUG = False


def build(causal: bool):
    nc = bacc.Bacc("TRN2", target_bir_lowering=False, debug=False,
                   num_devices=NCORES)

    def dt_in(n, s, d):
        return nc.dram_tensor(n, s, d, kind="ExternalInput").ap()

    g = {}
    g["xT_d"] = dt_in("xT", [E, T], F32)
    g["xrows_d"] = dt_in("xrows", [CH, E], F32)
    for w in ("wq", "wk", "wv", "wo"):
        g[w + "_d"] = dt_in(w, [E, CH], BF16)
    for v in ("swq", "swk", "swv", "swo", "qb", "kb", "vb", "ob"):
        g[v + "_d"] = dt_in(v, [CH], F32)
    g["tblk_d"] = dt_in("tblk", [128, 128], F32)
    g["ident_d"] = dt_in("ident", [128, 128], F32R)
    g["rvr_d"] = dt_in("rvr", [4, 512], F32)
    if not causal:
        g["mask_d"] = dt_in("mask", [T, T], F32)

    g["outT_d"] = nc.dram_tensor("outT", [CH, T], F32,
                                 kind="ExternalOutput").ap()
    if DEBUG:
        for nm, shp in (("dbg_qT", [128, T]), ("dbg_kT", [128, T]),
                        ("dbg_yv", [128, CH]), ("dbg_acc", [1, T]),
                        ("dbg_flags", [1, T]), ("dbg_ssel", [1, T]),
                        ("dbg_ctxT", [128, T]), ("dbg_sx", [1, T]),
                        ("dbg_vqi", [128, CH]), ("dbg_k2", [128, T])):
            g[nm] = nc.dram_tensor(nm, shp, F32, kind="ExternalOutput").ap()

    g["sx_in"] = nc.dram_tensor("sx_in", [CH], F32).ap()
    g["sx_out"] = nc.dram_tensor("sx_out", [T], F32, addr_space="Shared").ap()
    g["kv_in"] = nc.dram_tensor("kv_in", [2, T], F32).ap()
    g["kv_out"] = nc.dram_tensor("kv_out", [2, T], F32,
                                 addr_space="Shared").ap()
    g["acc_in"] = nc.dram_tensor("acc_in", [T], F32).ap()
    g["acc_out"] = nc.dram_tensor("acc_out", [T], F32,
                                  addr_space="Shared").ap()
    g["flg_b"] = nc.dram_tensor("flg_b", [T], F32).ap()
    g["sv_b"] = nc.dram_tensor("sv_b", [CH], F32).ap()
    g["warm_in"] = nc.dram_tensor("warm_in", [16], F32).ap()
    g["warm_out"] = nc.dram_tensor("warm_out", [NCORES * 16], F32,
                                   addr_space="Shared").ap()
    g["cmax_b"] = nc.dram_tensor("cmax_b", [T], F32).ap()
    g["ag1_in"] = nc.dram_tensor("ag1_in", [128 * T], F32).ap()
    g["ag1_out"] = nc.dram_tensor("ag1_out", [NCORES, 128 * T], F32,
                                  addr_space="Shared").ap()
    g["ag2_in"] = nc.dram_tensor("ag2_in", [128 * T], F32).ap()
    g["ag2_out"] = nc.dram_tensor("ag2_out", [NCORES, 128 * T], F32,
                                  addr_space="Shared").ap()
    g["sc_b"] = nc.dram_tensor("sc_b", [T], F32).ap()
    g["sselr_b"] = nc.dram_tensor("sselr_b", [T], F32).ap()
    g["rsselr_b"] = nc.dram_tensor("rsselr_b", [T], F32).ap()
    g["cmx_in"] = nc.dram_tensor("cmx_in", [T], F32).ap()
    g["cmx_out"] = nc.dram_tensor("cmx_out", [T], F32,
                                  addr_space="Shared").ap()
    g["rg"] = [list(range(NCORES))]

    with tile.TileContext(nc) as tc:
        _body(nc, tc, causal, g)
    nc.compile()
    return nc


def _body(nc, tc, causal, g):
    rg = g["rg"]

    def pool(name, bufs=1, space="SBUF"):
        cm = tc.tile_pool(name=name, bufs=bufs, space=space)
        return cm, cm.__enter__()

    per_cm, per = pool("per")

    tblk = per.tile([128, 128], F32, tag="tblk")
    ident = per.tile([128, 128], F32R, tag="ident")
    nc.sync.dma_start(tblk[:], g["tblk_d"][:])
    nc.sync.dma_start(ident[:], g["ident_d"][:])

    def load_pp(dram_vec, tag, p=per):
        t_ = p.tile([128, 2], F32, tag=tag)
        nc.sync.dma_start(t_[:], dram_vec.rearrange("(j p) -> p j", p=128))
        return t_

    swq_pp = load_pp(g["swq_d"], "swq")
    swk_pp = load_pp(g["swk_d"], "swk")
    swo_pp = load_pp(g["swo_d"], "swo")
    qb_pp = load_pp(g["qb_d"], "qb")
    kb_pp = load_pp(g["kb_d"], "kb")
    ob_pp = load_pp(g["ob_d"], "ob")
    qT = [per.tile([128, T], F32R, tag=f"qT{h}", name=f"qT{h}")
          for h in range(HL)]
    kTr = [per.tile([128, T], F32R, tag=f"kTr{h}", name=f"kTr{h}")
           for h in range(HL)]

    # long-lived mid tensors, stack-nested: per > kxp > vqp > yvp > wpool
    kx_cm, kxp = pool("kxp")
    kTx = [kxp.tile([128, T], F32, tag=f"kTx{h}", name=f"kTx{h}")
           for h in range(HL)]
    vq_cm, vqp = pool("vqp")
    vqi = vqp.tile([128, NT * CH], F32R, tag="vqi")
    yv_cm, yvp = pool("yvp")
    yv = yvp.tile([128, NT * CH], F32, tag="yv")

    # -------- w pool: xq + scale reps (P0 .. P1b) --------
    w_cm, wp = pool("wpool")
    swv_row = wp.tile([1, CH], F32, tag="swvr")
    vb_row = wp.tile([1, CH], F32, tag="vbr")
    nc.sync.dma_start(swv_row[:], g["swv_d"].rearrange("(a c) -> a c", a=1))
    nc.sync.dma_start(vb_row[:], g["vb_d"].rearrange("(a c) -> a c", a=1))
    swv_rep = wp.tile([128, CH], F32, tag="swvrep")
    vb_rep = wp.tile([128, CH], F32, tag="vbrep")
    nc.gpsimd.partition_broadcast(swv_rep[:], swv_row[:])
    nc.gpsimd.partition_broadcast(vb_rep[:], vb_row[:])

    # -------- P0: activation scales s_x --------
    p0_cm, p0 = pool("p0")
    xr = p0.tile([128, E], F32, tag="xr")
    am = p0.tile([128, 2], F32, tag="am")
    for j in range(2):
        nc.sync.dma_start(xr[:], g["xrows_d"][j * 128:(j + 1) * 128, :])
        nc.vector.tensor_reduce(am[:, j:j + 1], xr[:], axis=AX.X,
                                op=OP.max, apply_absolute_value=True)
    sxp = p0.tile([128, 2], F32, tag="sxp")
    nc.vector.tensor_scalar(sxp[:], am[:], EPS, 1.0 / Q8,
                            op0=OP.max, op1=OP.mult)
    nc.sync.dma_start(g["sx_in"].rearrange("(j p) -> p j", p=128), sxp[:])
    nc.gpsimd.collective_compute("AllGather", OP.bypass,
                                 ins=[g["sx_in"][:]], outs=[g["sx_out"][:]],
                                 replica_groups=rg)
    p0_cm.__exit__(None, None, None)

    sx_rep = wp.tile([128, T], F32, tag="sxrep")
    nc.sync.dma_start(sx_rep[:],
                      g["sx_out"].rearrange("(a t) -> a t", a=1)
                      .to_broadcast([128, T]))
    if DEBUG:
        nc.sync.dma_start(g["dbg_sx"][:], sx_rep[0:1, :])
    sx_pp = wp.tile([128, NT], F32, tag="sxpp")
    nc.scalar.dma_start(sx_pp[:], g["sx_out"].rearrange("(j p) -> p j", p=128))

    # -------- P1a: load + quantize x^T -> int bf16 --------
    xq = wp.tile([128, NE * T], BF16, tag="xq")
    p1a_cm, p1a = pool("p1a", bufs=3)
    rsx_rep = p1a.tile([128, T], F32, tag="rsxrep", bufs=1)
    nc.vector.reciprocal(rsx_rep[:], sx_rep[:])
    dma_engs = [nc.sync, nc.scalar, nc.gpsimd]
    rndc_pp = wp.tile([128, 1], F32, tag="rndcpp")
    nc.vector.memset(rndc_pp[:], RND_C)
    for et in range(NE):
        xt = p1a.tile([128, T], F32, tag="xt", bufs=3)
        dma_engs[et % 3].dma_start(xt[:], g["xT_d"][et * 128:(et + 1) * 128, :])
        nc.vector.tensor_tensor(xt[:], xt[:], rsx_rep[:], op=OP.mult)
        nc.vector.tensor_scalar(xt[:], xt[:], RND_C, None, op0=OP.add)
        nc.vector.tensor_scalar(xq[:, et * T:(et + 1) * T], xt[:],
                                RND_C, None, op0=OP.subtract)
    p1a_cm.__exit__(None, None, None)

    # -------- P1b: Q/K/V projections, three e-passes, streamed weights ----
    p1b_cm, p1b = pool("p1b", bufs=3)
    ps1_cm, ps1 = pool("ps1", space="PSUM")

    # Q pass: out^T [o, t] = wq_int^T.T @ xq  (+ scales/bias after)
    for name, wd, sw_pp, b_pp, outs, extra_scale in (
            ("q", g["wq_d"], swq_pp, qb_pp, qT, SCALING),
            ("k", g["wk_d"], swk_pp, kb_pp, kTx, None)):
        pP = [ps1.tile([128, T], F32, tag=f"pP{o}", name=f"pP{o}_{name}")
              for o in range(2)]
        for et in range(NE):
            we = p1b.tile([128, CH], BF16, tag="wstream")
            nc.sync.dma_start(we[:], wd[et * 128:(et + 1) * 128, :])
            xq_e = xq[:, et * T:(et + 1) * T]
            for o in range(2):
                for n in range(4):
                    nsl = slice(n * 512, (n + 1) * 512)
                    nc.tensor.matmul(pP[o][:, nsl],
                                     we[:, o * 128:(o + 1) * 128],
                                     xq_e[:, nsl],
                                     start=(et == 0), stop=(et == NE - 1))
        for o in range(2):
            e1 = p1b.tile([128, T], F32, tag="ev1", bufs=2)
            nc.vector.tensor_scalar(e1[:], pP[o][:], sw_pp[:, o:o + 1],
                                    None, op0=OP.mult)
            nc.vector.tensor_tensor(e1[:], e1[:], sx_rep[:], op=OP.mult)
            if extra_scale is not None:
                nc.vector.tensor_scalar(outs[o][:], e1[:], b_pp[:, o:o + 1],
                                        extra_scale, op0=OP.add, op1=OP.mult)
            else:
                nc.vector.tensor_scalar(outs[o][:], e1[:], b_pp[:, o:o + 1],
                                        None, op0=OP.add)
                nc.vector.tensor_copy(kTr[o][:], outs[o][:])

    ps1_cm.__exit__(None, None, None)
    # V pass: [t, o] = xq_block.T @ wv_int; one accumulation group per bank
    ps1v_cm, ps1v = pool("ps1v", bufs=2, space="PSUM")
    wv_res = p1b.tile([128, NE * CH], BF16, tag="wvres", bufs=1)
    for et in range(NE):
        nc.sync.dma_start(wv_res[:, et * CH:(et + 1) * CH],
                          g["wv_d"][et * 128:(et + 1) * 128, :])
    for j in range(NT):
        pVj = ps1v.tile([128, CH], F32, tag="pV", name=f"pV{j}")
        for et in range(NE):
            xq_e = xq[:, et * T:(et + 1) * T]
            nc.tensor.matmul(pVj[:],
                             xq_e[:, j * 128:(j + 1) * 128],
                             wv_res[:, et * CH:(et + 1) * CH],
                             start=(et == 0), stop=(et == NE - 1))
        ysl = slice(j * CH, (j + 1) * CH)
        e3 = p1b.tile([128, CH], F32, tag="ev3")
        nc.vector.tensor_scalar(e3[:], pVj[:], sx_pp[:, j:j + 1], None,
                                op0=OP.mult)
        nc.vector.tensor_tensor(e3[:], e3[:], swv_rep[:], op=OP.mult)
        nc.vector.tensor_tensor(yv[:, ysl], e3[:], vb_rep[:], op=OP.add)
    ps1v_cm.__exit__(None, None, None)
    p1b_cm.__exit__(None, None, None)
    w_cm.__exit__(None, None, None)

    if DEBUG:
        nc.sync.dma_start(g["dbg_qT"][:], qT[0][:].bitcast(F32))
        nc.sync.dma_start(g["dbg_kT"][:], kTx[0][:])
        nc.sync.dma_start(g["dbg_yv"][:], yv[:, 0:CH])

    # -------- P2/P3: k row-max + v out-quant row-max, AllReduce-max --------
    p23_cm, p23 = pool("p23", bufs=2)
    ps23_cm, ps23 = pool("ps23", bufs=2, space="PSUM")
    kmax_pp = p23.tile([128, NT], F32, tag="kmaxpp", bufs=1)
    for b in range(NT):
        for h in range(HL):
            psT = ps23.tile([128, 128], F32, tag="pstat", name=f"psk{b}_{h}")
            nc.tensor.transpose(psT[:], kTx[h][:, b * 128:(b + 1) * 128],
                                ident[:].bitcast(F32))
            if h == 0:
                nc.vector.tensor_reduce(kmax_pp[:, b:b + 1], psT[:],
                                        axis=AX.X, op=OP.max,
                                        apply_absolute_value=True)
            else:
                tmp_r = p23.tile([128, 1], F32, tag="tmpr")
                nc.vector.tensor_reduce(tmp_r[:], psT[:], axis=AX.X,
                                        op=OP.max, apply_absolute_value=True)
                nc.vector.tensor_tensor(kmax_pp[:, b:b + 1],
                                        kmax_pp[:, b:b + 1], tmp_r[:],
                                        op=OP.max)
    nc.sync.dma_start(g["kv_in"][0, :].rearrange("(j p) -> p j", p=128),
                      kmax_pp[:])

    ym = p23.tile([128, NT], F32, tag="ym")
    for j in range(NT):
        nc.vector.tensor_reduce(ym[:, j:j + 1], yv[:, j * CH:(j + 1) * CH],
                                axis=AX.X, op=OP.max,
                                apply_absolute_value=True)
    nc.sync.dma_start(g["kv_in"][1, :].rearrange("(j p) -> p j", p=128),
                      ym[:])
    nc.gpsimd.collective_compute("AllReduce", OP.max,
                                 ins=[g["kv_in"][:]], outs=[g["kv_out"][:]],
                                 replica_groups=rg)
    ps23_cm.__exit__(None, None, None)
    p23_cm.__exit__(None, None, None)

    def chunks_of(w):
        out, s = [], 0
        while s < w:
            out.append((s, min(512, w - s)))
            s += 512
        return out

    # -------- P5: pass 1 -> accumulated attention score --------
    p5_cm, p5 = pool("p5", bufs=3)
    p5m_cm, p5m = pool("p5m", bufs=2)
    psA_cm, psA_p = pool("psA", space="PSUM")
    psS_cm, psS_p = pool("psS", bufs=2, space="PSUM")
    pA = [psA_p.tile([1, 512], F32, tag=f"pA{n}", name=f"pA{n}") for n in range(4)]
    first_wr = [True] * 4
    for h in range(HL):
        for i in range(NT):
            c_cols = (i + 1) * 128 if causal else T
            nhalf = _cdiv(c_cols, 1024)
            if not causal:
                mrow = p5m.tile([128, T], F32, tag="mrow")
                nc.sync.dma_start(mrow[:], g["mask_d"][i * 128:(i + 1) * 128, :])
            halves = []
            for hf in range(nhalf):
                w = min(1024, c_cols - hf * 1024)
                psS = psS_p.tile([128, 1024], F32, tag="pS")
                for (s0, wd) in chunks_of(w):
                    nc.tensor.matmul(
                        psS[:, s0:s0 + wd],
                        qT[h][:, i * 128:(i + 1) * 128],
                        kTr[h][:, hf * 1024 + s0:hf * 1024 + s0 + wd],
                        start=True, stop=True)
                halves.append((psS, w))
            if causal:
                hfd = (i * 128) // 1024
                dge = i * 128 - hfd * 1024
                psD = halves[hfd][0]
                nc.vector.tensor_tensor(psD[:, dge:dge + 128],
                                        psD[:, dge:dge + 128], tblk[:],
                                        op=OP.add)
            else:
                for hf, (psS, w) in enumerate(halves):
                    nc.vector.tensor_tensor(psS[:, :w], psS[:, :w],
                                            mrow[:, hf * 1024:hf * 1024 + w],
                                            op=OP.add)
            zz = p5.tile([128, 2], F32, tag="zz")
            pp = []
            for hf, (psS, w) in enumerate(halves):
                p1t = p5.tile([128, 1024], F32R, tag="p1t")
                nc.scalar.activation(p1t[:, :w], psS[:, :w], ACTF.Exp,
                                     bias=0.0, scale=1.0,
                                     accum_out=zz[:, hf:hf + 1])
                wpad = _cdiv(w, 512) * 512
                if wpad > w:
                    nc.vector.memset(p1t[:, w:wpad].bitcast(F32), 0.0)
                pp.append(p1t)
            z = p5.tile([128, 1], F32, tag="z")
            if nhalf == 1:
                nc.vector.tensor_copy(z[:], zz[:, 0:1])
            else:
                nc.vector.tensor_reduce(z[:], zz[:, :nhalf], axis=AX.X,
                                        op=OP.add)
            rz = p5.tile([128, 1], F32R, tag="rz")
            with nc.allow_low_precision(reason="fp32r matmul feed"):
                nc.vector.reciprocal(rz[:], z[:])
            for n in range(_cdiv(c_cols, 512)):
                hf = (n * 512) // 1024
                off = n * 512 - hf * 1024
                nc.tensor.matmul(pA[n][:], rz[:], pp[hf][:, off:off + 512],
                                 start=first_wr[n],
                                 stop=(h == HL - 1 and i == NT - 1))
                first_wr[n] = False
    accs = p5.tile([1, T], F32, tag="accs")
    for n in range(4):
        nc.vector.tensor_copy(accs[:, n * 512:(n + 1) * 512], pA[n][:])
    nc.sync.dma_start(g["acc_in"].rearrange("(a t) -> a t", a=1), accs[:])
    nc.gpsimd.collective_compute("AllReduce", OP.add,
                                 ins=[g["acc_in"][:]], outs=[g["acc_out"][:]],
                                 replica_groups=rg)
    psS_cm.__exit__(None, None, None)
    psA_cm.__exit__(None, None, None)
    p5m_cm.__exit__(None, None, None)
    p5_cm.__exit__(None, None, None)

    # -------- P4: v quantization --------
    sv_pp = per.tile([128, HL], F32, tag="svpp")
    p4_cm, p4 = pool("p4", bufs=2)
    vq1 = p4.tile([128, NT * CH], F32, tag="vq1")
    ymf = p4.tile([128, NT], F32, tag="ymf")
    nc.sync.dma_start(ymf[:], g["kv_out"][1, :].rearrange("(j p) -> p j",
                                                          p=128))
    sy = p4.tile([128, NT], F32, tag="sy")
    nc.vector.tensor_scalar(sy[:], ymf[:], EPS, 1.0 / Q8,
                            op0=OP.max, op1=OP.mult)
    rsy = p4.tile([128, NT], F32, tag="rsy")
    nc.vector.reciprocal(rsy[:], sy[:])
    for j in range(NT):
        ysl = slice(j * CH, (j + 1) * CH)
        d1 = p4.tile([128, CH], F32, tag="d1")
        nc.vector.tensor_scalar(d1[:], yv[:, ysl], rsy[:, j:j + 1], RND_C,
                                op0=OP.mult, op1=OP.add)
        nc.vector.tensor_scalar(vq1[:, ysl], d1[:], RND_C, sy[:, j:j + 1],
                                op0=OP.subtract, op1=OP.mult)
    ps4_cm, ps4 = pool("ps4", bufs=2, space="PSUM")
    vmax_pp = p4.tile([128, HL], F32, tag="vmaxpp", bufs=1)
    for j in range(NT):
        for ob in range(HL):
            psT = ps4.tile([128, 128], F32, tag="pstat4", name=f"psv{j}_{ob}")
            nc.tensor.transpose(
                psT[:], vq1[:, j * CH + ob * 128:j * CH + (ob + 1) * 128],
                ident[:].bitcast(F32))
            if j == 0:
                nc.vector.tensor_reduce(vmax_pp[:, ob:ob + 1], psT[:],
                                        axis=AX.X, op=OP.max,
                                        apply_absolute_value=True)
            else:
                tmp_r4 = p4.tile([128, 1], F32, tag="tmpr4")
                nc.vector.tensor_reduce(tmp_r4[:], psT[:], axis=AX.X,
                                        op=OP.max, apply_absolute_value=True)
                nc.vector.tensor_tensor(vmax_pp[:, ob:ob + 1],
                                        vmax_pp[:, ob:ob + 1], tmp_r4[:],
                                        op=OP.max)
    ps4_cm.__exit__(None, None, None)
    nc.vector.tensor_scalar(sv_pp[:], vmax_pp[:], EPS, 1.0 / Q8,
                            op0=OP.max, op1=OP.mult)
    nc.sync.dma_start(g["sv_b"].rearrange("(h p) -> p h", p=128), sv_pp[:])
    sv_row = p4.tile([1, CH], F32, tag="svrow")
    nc.sync.dma_start(sv_row[:], g["sv_b"].rearrange("(a c) -> a c", a=1))
    rsv_row = p4.tile([1, CH], F32, tag="rsvrow")
    nc.vector.reciprocal(rsv_row[:], sv_row[:])
    rsv_rep = p4.tile([128, CH], F32, tag="rsvrep")
    nc.gpsimd.partition_broadcast(rsv_rep[:], rsv_row[:])
    for j in range(NT):
        ysl = slice(j * CH, (j + 1) * CH)
        m1 = p4.tile([128, CH], F32, tag="m1")
        nc.vector.tensor_tensor(m1[:], vq1[:, ysl], rsv_rep[:], op=OP.mult)
        nc.vector.tensor_scalar(m1[:], m1[:], RND_C, None, op0=OP.add)
        nc.vector.tensor_scalar(vqi[:, ysl], m1[:], RND_C, None,
                                op0=OP.subtract)
    p4_cm.__exit__(None, None, None)
    yv_cm.__exit__(None, None, None)


    # -------- P6: top-k flags + k scale selection + k2 ints --------
    sel_cm, selp = pool("selp")
    ssel_rep = selp.tile([128, T], F32, tag="sselrep")
    p6_cm, p6 = pool("p6")
    acc4 = p6.tile([4, 512], F32, tag="acc4")
    nc.sync.dma_start(acc4[:], g["acc_out"].rearrange("(r s) -> r s", r=4))
    if DEBUG:
        nc.sync.dma_start(g["dbg_acc"].rearrange("a (r s) -> (a r) s", r=4),
                          acc4[:])
    rvr = p6.tile([4, 512], F32, tag="rvr")
    nc.sync.dma_start(rvr[:], g["rvr_d"][:])
    nc.vector.tensor_tensor(acc4[:], acc4[:], rvr[:], op=OP.mult)
    nc.vector.tensor_scalar(acc4[:], acc4[:], 1.0 / H, None, op0=OP.mult)
    tkw = p6.tile([4, 512], F32, tag="tkw")
    ton = acc4[:]
    for k_on in range(0, K_TOP, 8):
        k_this = min(k_on + 8, K_TOP) - k_on
        mx8 = p6.tile([4, 8], F32, tag="mx8")
        nc.vector.max(out=mx8[:], in_=ton)
        if k_this < 8:
            nc.vector.memset(mx8[:, k_this:], 0)
        nc.vector.match_replace(out=tkw[:], in_to_replace=mx8[:],
                                in_values=ton, imm_value=0)
        ton = tkw[:]
    nc.vector.tensor_sub(out=tkw[:], in0=acc4[:], in1=tkw[:])
    nc.vector.tensor_scalar(tkw[:], tkw[:], 0.0, None, op0=OP.is_gt)
    nc.sync.dma_start(g["flg_b"].rearrange("(r s) -> r s", r=4), tkw[:])
    flags = p6.tile([1, T], F32, tag="flags")
    nc.sync.dma_start(flags[:], g["flg_b"].rearrange("(a t) -> a t", a=1))
    if DEBUG:
        nc.sync.dma_start(g["dbg_flags"][:], flags[:])
    flags_i = p6.tile([1, T], mybir.dt.int32, tag="flagsi")
    nc.vector.tensor_scalar(flags_i[:], flags[:], 0.0, None, op0=OP.is_gt)

    kmaxf = p6.tile([1, T], F32, tag="kmaxf")
    nc.sync.dma_start(kmaxf[:], g["kv_out"][0:1, :])
    s8 = p6.tile([1, T], F32, tag="s8")
    nc.vector.tensor_scalar(s8[:], kmaxf[:], 1.0 / Q8, EPS,
                            op0=OP.mult, op1=OP.max)
    ssel = p6.tile([1, T], F32, tag="ssel")
    nc.vector.tensor_scalar(ssel[:], kmaxf[:], EPS, 1.0 / Q4,
                            op0=OP.max, op1=OP.mult)
    nc.vector.copy_predicated(ssel[:], flags_i[:], s8[:])
    rssel = p6.tile([1, T], F32, tag="rssel")
    nc.vector.reciprocal(rssel[:], ssel[:])
    rssel_rep = p6.tile([128, T], F32, tag="rsselrep")
    nc.sync.dma_start(g["sselr_b"].rearrange("(a t) -> a t", a=1), ssel[:])
    nc.sync.dma_start(g["rsselr_b"].rearrange("(a t) -> a t", a=1), rssel[:])
    nc.sync.dma_start(ssel_rep[:],
                      g["sselr_b"].rearrange("(a t) -> a t", a=1)
                      .to_broadcast([128, T]))
    nc.sync.dma_start(rssel_rep[:],
                      g["rsselr_b"].rearrange("(a t) -> a t", a=1)
                      .to_broadcast([128, T]))
    if DEBUG:
        nc.sync.dma_start(g["dbg_ssel"][:], ssel[:])
    for h in range(HL):
        kq = p6.tile([128, T], F32, tag="kq")
        nc.vector.tensor_tensor(kq[:], kTx[h][:], rssel_rep[:], op=OP.mult)
        nc.vector.tensor_scalar(kq[:], kq[:], RND_C, None, op0=OP.add)
        nc.vector.tensor_scalar(kTr[h][:], kq[:], RND_C, None,
                                op0=OP.subtract)
    p6_cm.__exit__(None, None, None)

    if DEBUG:
        nc.sync.dma_start(g["dbg_k2"][:], kTr[0][:].bitcast(F32))
        nc.sync.dma_start(g["dbg_vqi"][:], vqi[:, 0:CH].bitcast(F32))

    # -------- P7: pass 2 attention --------
    ctx_cm, ctxp = pool("ctxp")
    ctxT = [ctxp.tile([128, T], F32, tag=f"ctxT{h}", name=f"ctxT{h}") for h in range(HL)]
    psb_cm, psbp = pool("psbp")
    psb = psbp.tile([128, NT * 512], F32R, tag="psb")
    p7_cm, p7 = pool("p7", bufs=3)
    cmax_pp = p7.tile([128, NT], F32, tag="cmaxpp", bufs=1)
    p7m_cm, p7m = pool("p7m", bufs=2)
    ps7s_cm, ps7S = pool("ps7s", bufs=2, space="PSUM")
    ps7t_cm, ps7T = pool("ps7t", bufs=2, space="PSUM")
    ps7c_cm, ps7C = pool("ps7c", bufs=2, space="PSUM")
    for h in range(HL):
        for i in range(NT):
            c_cols = (i + 1) * 128 if causal else T
            nhalf = _cdiv(c_cols, 1024)
            if not causal:
                mrow = p7m.tile([128, T], F32, tag="mrow2")
                nc.sync.dma_start(mrow[:], g["mask_d"][i * 128:(i + 1) * 128, :])
            halves = []
            for hf in range(nhalf):
                w = min(1024, c_cols - hf * 1024)
                psS = ps7S.tile([128, 1024], F32, tag="pS2")
                for (s0, wd) in chunks_of(w):
                    nc.tensor.matmul(
                        psS[:, s0:s0 + wd],
                        qT[h][:, i * 128:(i + 1) * 128],
                        kTr[h][:, hf * 1024 + s0:hf * 1024 + s0 + wd],
                        start=True, stop=True)
                nc.vector.tensor_tensor(psS[:, :w], psS[:, :w],
                                        ssel_rep[:, hf * 1024:hf * 1024 + w],
                                        op=OP.mult)
                halves.append((psS, w))
            if causal:
                hfd = (i * 128) // 1024
                dge = i * 128 - hfd * 1024
                psD = halves[hfd][0]
                nc.vector.tensor_tensor(psD[:, dge:dge + 128],
                                        psD[:, dge:dge + 128], tblk[:],
                                        op=OP.add)
            else:
                for hf, (psS, w) in enumerate(halves):
                    nc.vector.tensor_tensor(psS[:, :w], psS[:, :w],
                                            mrow[:, hf * 1024:hf * 1024 + w],
                                            op=OP.add)
            zz = p7.tile([128, 2], F32, tag="zz2")
            pp = []
            for hf, (psS, w) in enumerate(halves):
                p2t = p7.tile([128, 1024], F32, tag="p2t")
                nc.scalar.activation(p2t[:, :w], psS[:, :w], ACTF.Exp,
                                     bias=0.0, scale=1.0,
                                     accum_out=zz[:, hf:hf + 1])
                pp.append(p2t)
            z = p7.tile([128, 1], F32, tag="z2")
            if nhalf == 1:
                nc.vector.tensor_copy(z[:], zz[:, 0:1])
            else:
                nc.vector.tensor_reduce(z[:], zz[:, :nhalf], axis=AX.X,
                                        op=OP.add)
            rz = p7.tile([128, 1], F32, tag="rz2")
            nc.vector.reciprocal(rz[:], z[:])
            pn = []
            for hf, (psS, w) in enumerate(halves):
                p2n = p7.tile([128, 1024], F32R, tag="p2n")
                nc.vector.tensor_scalar(p2n[:, :w], pp[hf][:, :w], rz[:],
                                        None, op0=OP.mult)
                pn.append(p2n)
            # psb layout: i-block-major — block ib at ib*(NT*128), s-tile j
            # at col j*128 within it. Quad transposes -> ONE 512-wide copy.
            nj = (i + 1) if causal else NT
            ib = i % 4
            for jq in range(0, nj, 4):
                jn = min(4, nj - jq)
                psT = ps7T.tile([128, 512], F32, tag="pT")
                for jj in range(jn):
                    j = jq + jj
                    hf = (j * 128) // 1024
                    off = j * 128 - hf * 1024
                    nc.tensor.transpose(
                        psT[:, jj * 128:(jj + 1) * 128].bitcast(F32R),
                        pn[hf][:, off:off + 128], ident[:])
                dst = psb[:, ib * (NT * 128) + jq * 128:
                          ib * (NT * 128) + (jq + jn) * 128]
                if (jq // 4) % 2 == 0:
                    nc.scalar.copy(dst, psT[:, :jn * 128])
                else:
                    nc.vector.tensor_copy(dst, psT[:, :jn * 128])
            if i % 4 == 3:
                sb = i // 4
                jmax = (4 * sb + 4) if causal else NT
                if causal:
                    for j in range(4 * sb + 1, jmax):
                        for ibg in range(j - 4 * sb):
                            nc.vector.memset(
                                psb[:, ibg * (NT * 128) + j * 128:
                                    ibg * (NT * 128) +
                                    (j + 1) * 128].bitcast(F32), 0.0)
                psb3 = psb[:].rearrange("p (ib jt) -> p ib jt", ib=4)
                psC = ps7C.tile([128, 512], F32, tag="pC")
                for j in range(jmax):
                    nc.tensor.matmul(
                        psC[:],
                        vqi[:, j * CH + h * 128:j * CH + (h + 1) * 128],
                        psb3[:, :, j * 128:(j + 1) * 128],
                        start=(j == 0), stop=(j == jmax - 1))
                nc.vector.tensor_scalar(ctxT[h][:, sb * 512:(sb + 1) * 512],
                                        psC[:], sv_pp[:, h:h + 1], None,
                                        op0=OP.mult)
        # head complete: ctx absmax partial + shard DMA + per-head AllGather
        agi = g["ag1_in"] if h == 0 else g["ag2_in"]
        nc.sync.dma_start(
            agi[:].rearrange("(p t) -> p t", p=128), ctxT[h][:])
        for bq in range(0, NT, 4):
            psT = ps7T.tile([128, 512], F32, tag="pT", name=f"pTs{h}_{bq}")
            for bb in range(4):
                b = bq + bb
                nc.tensor.transpose(psT[:, bb * 128:(bb + 1) * 128],
                                    ctxT[h][:, b * 128:(b + 1) * 128]
                                    .bitcast(F32),
                                    ident[:].bitcast(F32))
            for bb in range(4):
                b = bq + bb
                if h == 0:
                    nc.vector.tensor_reduce(
                        cmax_pp[:, b:b + 1], psT[:, bb * 128:(bb + 1) * 128],
                        axis=AX.X, op=OP.max, apply_absolute_value=True)
                else:
                    tmp_r8 = p7.tile([128, 1], F32, tag="tmpr8")
                    nc.vector.tensor_reduce(
                        tmp_r8[:], psT[:, bb * 128:(bb + 1) * 128],
                        axis=AX.X, op=OP.max, apply_absolute_value=True)
                    nc.vector.tensor_tensor(cmax_pp[:, b:b + 1],
                                            cmax_pp[:, b:b + 1], tmp_r8[:],
                                            op=OP.max)
        if h == 0:
            nc.gpsimd.collective_compute(
                "AllGather", OP.bypass, ins=[g["ag1_in"][:]],
                outs=[g["ag1_out"][:]], replica_groups=rg)
        else:
            nc.sync.dma_start(
                g["cmx_in"].rearrange("(j p) -> p j", p=128), cmax_pp[:])
            ar_i = nc.gpsimd.collective_compute(
                "AllReduce", OP.max, ins=[g["cmx_in"][:]],
                outs=[g["cmx_out"][:]], replica_groups=rg)
            ag2_i = nc.gpsimd.collective_compute(
                "AllGather", OP.bypass, ins=[g["ag2_in"][:]],
                outs=[g["ag2_out"][:]], replica_groups=rg)
            add_dep_helper(ag2_i.ins, ar_i.ins, True, "cmax AR before ctx AG2")
    ps7c_cm.__exit__(None, None, None)
    ps7t_cm.__exit__(None, None, None)
    ps7s_cm.__exit__(None, None, None)
    p7m_cm.__exit__(None, None, None)
    p7_cm.__exit__(None, None, None)
    psb_cm.__exit__(None, None, None)

    if DEBUG:
        nc.sync.dma_start(g["dbg_ctxT"][:], ctxT[0][:])

    ctx_cm.__exit__(None, None, None)
    sel_cm.__exit__(None, None, None)
    vq_cm.__exit__(None, None, None)
    kx_cm.__exit__(None, None, None)

    # -------- P9: output projection --------
    p9p_cm, p9p = pool("p9p")
    p9_cm, p9 = pool("p9", bufs=2)
    cmax = p9p.tile([1, T], F32, tag="cmax")
    nc.sync.dma_start(cmax[:], g["cmx_out"].rearrange("(a t) -> a t", a=1))
    sc = p9p.tile([1, T], F32, tag="sc")
    nc.vector.tensor_scalar(sc[:], cmax[:], EPS, 1.0 / Q8,
                            op0=OP.max, op1=OP.mult)
    nc.sync.dma_start(g["sc_b"].rearrange("(a t) -> a t", a=1), sc[:])
    sc_rep = p9p.tile([128, T], F32, tag="screp")
    rsc_rep = p9p.tile([128, T], F32, tag="rscrep")
    nc.sync.dma_start(sc_rep[:],
                      g["sc_b"].rearrange("(a t) -> a t", a=1)
                      .to_broadcast([128, T]))
    nc.vector.reciprocal(rsc_rep[:], sc_rep[:])
    rndc9 = p9p.tile([128, 1], F32, tag="rndc9")
    nc.vector.memset(rndc9[:], RND_C)

    wo_sb = p9p.tile([128, NE * CH], BF16, tag="wo")
    for et in range(NE):
        nc.sync.dma_start(wo_sb[:, et * CH:(et + 1) * CH],
                          g["wo_d"][et * 128:(et + 1) * 128, :])

    ps9_cm, ps9 = pool("ps9", space="PSUM")
    pO = [ps9.tile([128, T], F32, tag=f"pO{o}", name=f"pO{o}") for o in range(2)]
    et_order = [e for e in range(NE) if e % 2 == 0] + \
               [e for e in range(NE) if e % 2 == 1]
    for eti, et in enumerate(et_order):
        c0 = et // 2
        ct = p9.tile([128, T], F32, tag="ct", bufs=3)
        ag_src = g["ag1_out"] if (et % 2 == 0) else g["ag2_out"]
        (nc.sync if eti % 2 == 0 else nc.gpsimd).dma_start(
            ct[:], ag_src[c0, :].rearrange("(p t) -> p t", p=128))
        q1 = p9.tile([128, T], F32, tag="q1", bufs=3)
        nc.vector.tensor_tensor(q1[:], ct[:], rsc_rep[:], op=OP.mult)
        nc.scalar.activation(q1[:], q1[:], ACTF.Identity, bias=rndc9[:])
        co = p9.tile([128, T], BF16, tag="co", bufs=3)
        nc.vector.tensor_scalar(co[:], q1[:], RND_C, None, op0=OP.subtract)
        for o in range(2):
            wsl = slice(et * CH + o * 128, et * CH + (o + 1) * 128)
            for n in range(4):
                nsl = slice(n * 512, (n + 1) * 512)
                nc.tensor.matmul(pO[o][:, nsl], wo_sb[:, wsl], co[:, nsl],
                                 start=(eti == 0), stop=(eti == NE - 1))
    for o in range(2):
        f1 = p9.tile([128, T], F32, tag="f1", bufs=1)
        nc.vector.tensor_scalar(f1[:], pO[o][:], swo_pp[:, o:o + 1], None,
                                op0=OP.mult)
        nc.vector.tensor_tensor(f1[:], f1[:], sc_rep[:], op=OP.mult)
        outsb = p9.tile([128, T], F32, tag="outsb", bufs=1)
        nc.vector.tensor_scalar(outsb[:], f1[:], ob_pp[:, o:o + 1], None,
                                op0=OP.add)
        nc.sync.dma_start(g["outT_d"][o * 128:(o + 1) * 128, :], outsb[:])
    ps9_cm.__exit__(None, None, None)
    p9_cm.__exit__(None, None, None)
    p9p_cm.__exit__(None, None, None)
    per_cm.__exit__(None, None, None)


# ==================== host side ====================

_CACHE = {}


def _get_nc(causal):
    if causal not in _CACHE:
        _CACHE[causal] = build(causal)
    return _CACHE[causal]


def _quant_w(w):
    amax = np.max(np.abs(w), axis=-1, keepdims=True)
    s = np.maximum(amax, np.float32(EPS)) / np.float32(Q8)
    wi = np.round((w / s).astype(np.float32))
    return wi, s[:, 0].astype(np.float32)


def kernel(hidden_states, attention_mask, q_w, q_b, k_w, k_b, v_w, v_b,
           o_w, o_b, num_heads):
    hidden_states = np.asarray(hidden_states, dtype=np.float32)
    attention_mask = np.asarray(attention_mask, dtype=np.float32)
    assert int(num_heads) == H
    B, T_, E_ = hidden_states.shape
    assert (B, T_, E_) == (1, T, E)

    x = np.ascontiguousarray(hidden_states[0])        # [T, E]
    xT = np.ascontiguousarray(x.T)                    # [E, T]

    causal_ref = np.triu(np.full((T, T), np.float32(NEG), np.float32), k=1)
    mfull = np.ascontiguousarray(attention_mask[0, 0])
    causal = bool(np.array_equal(mfull, causal_ref))

    nc = _get_nc(causal)

    wqi, sq = _quant_w(np.asarray(q_w, np.float32))
    wki, sk = _quant_w(np.asarray(k_w, np.float32))
    wvi, sv = _quant_w(np.asarray(v_w, np.float32))
    woi, so = _quant_w(np.asarray(o_w, np.float32))

    tblk = np.triu(np.full((128, 128), np.float32(NEG), np.float32), k=1)
    ident = np.eye(128, dtype=np.float32)
    rowvec = np.float32(T) - np.arange(T, dtype=np.float32)
    rvr = (np.float32(1.0) / rowvec).reshape(4, 512).astype(np.float32)

    in_maps = []
    for c in range(NCORES):
        ch = slice(c * CH, (c + 1) * CH)
        im = dict(
            xT=xT,
            xrows=np.ascontiguousarray(x[ch, :]),
            wq=np.ascontiguousarray(wqi[ch, :].T).astype(ml_dtypes.bfloat16),
            wk=np.ascontiguousarray(wki[ch, :].T).astype(ml_dtypes.bfloat16),
            wv=np.ascontiguousarray(wvi[ch, :].T).astype(ml_dtypes.bfloat16),
            wo=np.ascontiguousarray(woi[ch, :].T).astype(ml_dtypes.bfloat16),
            swq=np.ascontiguousarray(sq[ch]),
            swk=np.ascontiguousarray(sk[ch]),
            swv=np.ascontiguousarray(sv[ch]),
            swo=np.ascontiguousarray(so[ch]),
            qb=np.ascontiguousarray(np.asarray(q_b, np.float32)[ch]),
            kb=np.ascontiguousarray(np.asarray(k_b, np.float32)[ch]),
            vb=np.ascontiguousarray(np.asarray(v_b, np.float32)[ch]),
            ob=np.ascontiguousarray(np.asarray(o_b, np.float32)[ch]),
            tblk=tblk, ident=ident, rvr=rvr,
        )
        if not causal:
            im["mask"] = mfull
        in_maps.append(im)

    res = run_bass_kernel_spmd(nc, in_maps, list(range(NCORES)))
    kernel.last_results = res.results
    out = np.empty((T, E), dtype=np.float32)
    for c in range(NCORES):
        out[:, c * CH:(c + 1) * CH] = res.results[c]["outT"].T
    return out.reshape(1, T, E)


# revision 31
# speedup vs baseline: 1.0522x; 1.0522x over previous
"""CalScaleOPTAttention on 8 TRN2 NeuronCores.

Sharding: heads across cores (2 heads / core, 256 channels each).
Device-side compute keeps all quantization arithmetic; quantized values are
small integers, exact in bf16, so projection matmuls run as int-bf16 with the
rank-1 scale factors applied after the matmul (more accurate than fp32).
Attention matmuls (unquantized q/k, softmax probs) use fp32r (TF32-like,
11-bit mantissa). Causal masking is exploited structurally: column blocks
beyond the diagonal are never computed (exp == 0 exactly).

Collectives: AllGather(act scales), AllReduce-max(k/v quant stats),
AllReduce-add(accumulated attention score), AllGather(ctx + ctx absmax).
"""

import numpy as np
import ml_dtypes

import concourse.bass as bass
import concourse.mybir as mybir
import concourse.tile as tile
from concourse.tile import add_dep_helper
from concourse import bacc
from concourse.bass_utils import run_bass_kernel_spmd

F32 = mybir.dt.float32
F32R = mybir.dt.float32r
BF16 = mybir.dt.bfloat16
AX = mybir.AxisListType
OP = mybir.AluOpType
ACTF = mybir.ActivationFunctionType

NCORES = 8
T = 2048
E = 2048
H = 16
D = 128                   # head dim
HL = H // NCORES          # heads per core = 2
CH = HL * D               # channels per core = 256
NT = T // 128             # 16 row tiles
NE = E // 128             # 16 contraction tiles
Q8 = 127.0
Q4 = 7.0
EPS = 1e-5
NEG = -1e9
RND_C = 12582912.0        # 1.5 * 2**23 round-to-int trick constant
SCALING = float(D) ** -0.5
K_TOP = T // 40           # 51
AGW = CH * T + T          # ctx allgather row width per core


def _cdiv(a, b):
    return (a + b - 1) // b


DEBUG = False


def build(causal: bool):
    nc = bacc.Bacc("TRN2", target_bir_lowering=False, debug=False,
                   num_devices=NCORES)

    def dt_in(n, s, d):
        return nc.dram_tensor(n, s, d, kind="ExternalInput").ap()

    g = {}
    g["xT_d"] = dt_in("xT", [E, T], F32)
    g["xrows_d"] = dt_in("xrows", [CH, E], F32)
    for w in ("wq", "wk", "wv", "wo"):
        g[w + "_d"] = dt_in(w, [E, CH], BF16)
    for v in ("swq", "swk", "swv", "swo", "qb", "kb", "vb", "ob"):
        g[v + "_d"] = dt_in(v, [CH], F32)
    g["tblk_d"] = dt_in("tblk", [128, 128], F32)
    g["ident_d"] = dt_in("ident", [128, 128], F32R)
    g["rvr_d"] = dt_in("rvr", [4, 512], F32)
    if not causal:
        g["mask_d"] = dt_in("mask", [T, T], F32)

    g["outT_d"] = nc.dram_tensor("outT", [CH, T], F32,
                                 kind="ExternalOutput").ap()
    if DEBUG:
        for nm, shp in (("dbg_qT", [128, T]), ("dbg_kT", [128, T]),
                        ("dbg_yv", [128, CH]), ("dbg_acc", [1, T]),
                        ("dbg_flags", [1, T]), ("dbg_ssel", [1, T]),
                        ("dbg_ctxT", [128, T]), ("dbg_sx", [1, T]),
                        ("dbg_vqi", [128, CH]), ("dbg_k2", [128, T])):
            g[nm] = nc.dram_tensor(nm, shp, F32, kind="ExternalOutput").ap()

    g["sx_in"] = nc.dram_tensor("sx_in", [CH], F32).ap()
    g["sx_out"] = nc.dram_tensor("sx_out", [T], F32, addr_space="Shared").ap()
    g["kv_in"] = nc.dram_tensor("kv_in", [2, T], F32).ap()
    g["kv_out"] = nc.dram_tensor("kv_out", [2, T], F32,
                                 addr_space="Shared").ap()
    g["acc_in"] = nc.dram_tensor("acc_in", [T], F32).ap()
    g["acc_out"] = nc.dram_tensor("acc_out", [T], F32,
                                  addr_space="Shared").ap()
    g["flg_b"] = nc.dram_tensor("flg_b", [T], F32).ap()
    g["sv_b"] = nc.dram_tensor("sv_b", [CH], F32).ap()
    g["warm_in"] = nc.dram_tensor("warm_in", [16], F32).ap()
    g["warm_out"] = nc.dram_tensor("warm_out", [NCORES * 16], F32,
                                   addr_space="Shared").ap()
    g["cmax_b"] = nc.dram_tensor("cmax_b", [T], F32).ap()
    g["ag1_in"] = nc.dram_tensor("ag1_in", [128 * T], F32).ap()
    g["ag1_out"] = nc.dram_tensor("ag1_out", [NCORES, 128 * T], F32,
                                  addr_space="Shared").ap()
    g["ag2_in"] = nc.dram_tensor("ag2_in", [128 * T], F32).ap()
    g["ag2_out"] = nc.dram_tensor("ag2_out", [NCORES, 128 * T], F32,
                                  addr_space="Shared").ap()
    g["sc_b"] = nc.dram_tensor("sc_b", [T], F32).ap()
    g["sselr_b"] = nc.dram_tensor("sselr_b", [T], F32).ap()
    g["rsselr_b"] = nc.dram_tensor("rsselr_b", [T], F32).ap()
    g["cmx_in"] = nc.dram_tensor("cmx_in", [T], F32).ap()
    g["cmx_out"] = nc.dram_tensor("cmx_out", [T], F32,
                                  addr_space="Shared").ap()
    g["rg"] = [list(range(NCORES))]

    with tile.TileContext(nc) as tc:
        _body(nc, tc, causal, g)
    nc.compile()
    return nc


def _body(nc, tc, causal, g):
    rg = g["rg"]

    def pool(name, bufs=1, space="SBUF"):
        cm = tc.tile_pool(name=name, bufs=bufs, space=space)
        return cm, cm.__enter__()

    per_cm, per = pool("per")

    tblk = per.tile([128, 128], F32, tag="tblk")
    ident = per.tile([128, 128], F32R, tag="ident")
    nc.sync.dma_start(tblk[:], g["tblk_d"][:])
    nc.sync.dma_start(ident[:], g["ident_d"][:])

    def load_pp(dram_vec, tag, p=per):
        t_ = p.tile([128, 2], F32, tag=tag)
        nc.sync.dma_start(t_[:], dram_vec.rearrange("(j p) -> p j", p=128))
        return t_

    swq_pp = load_pp(g["swq_d"], "swq")
    swk_pp = load_pp(g["swk_d"], "swk")
    swo_pp = load_pp(g["swo_d"], "swo")
    qb_pp = load_pp(g["qb_d"], "qb")
    kb_pp = load_pp(g["kb_d"], "kb")
    ob_pp = load_pp(g["ob_d"], "ob")
    qT = [per.tile([128, T], F32R, tag=f"qT{h}", name=f"qT{h}")
          for h in range(HL)]
    kTr = [per.tile([128, T], F32R, tag=f"kTr{h}", name=f"kTr{h}")
           for h in range(HL)]

    # long-lived mid tensors, stack-nested: per > kxp > vqp > yvp > wpool
    kx_cm, kxp = pool("kxp")
    kTx = [kxp.tile([128, T], F32, tag=f"kTx{h}", name=f"kTx{h}")
           for h in range(HL)]
    vq_cm, vqp = pool("vqp")
    vqi = vqp.tile([128, NT * CH], F32R, tag="vqi")
    yv_cm, yvp = pool("yvp")
    yv = yvp.tile([128, NT * CH], F32, tag="yv")

    # -------- w pool: xq + scale reps (P0 .. P1b) --------
    w_cm, wp = pool("wpool")
    swv_row = wp.tile([1, CH], F32, tag="swvr")
    vb_row = wp.tile([1, CH], F32, tag="vbr")
    nc.sync.dma_start(swv_row[:], g["swv_d"].rearrange("(a c) -> a c", a=1))
    nc.sync.dma_start(vb_row[:], g["vb_d"].rearrange("(a c) -> a c", a=1))
    swv_rep = wp.tile([128, CH], F32, tag="swvrep")
    vb_rep = wp.tile([128, CH], F32, tag="vbrep")
    nc.gpsimd.partition_broadcast(swv_rep[:], swv_row[:])
    nc.gpsimd.partition_broadcast(vb_rep[:], vb_row[:])

    # -------- P0: activation scales s_x --------
    p0_cm, p0 = pool("p0")
    xr = p0.tile([128, E], F32, tag="xr")
    am = p0.tile([128, 2], F32, tag="am")
    for j in range(2):
        nc.sync.dma_start(xr[:], g["xrows_d"][j * 128:(j + 1) * 128, :])
        nc.vector.tensor_reduce(am[:, j:j + 1], xr[:], axis=AX.X,
                                op=OP.max, apply_absolute_value=True)
    sxp = p0.tile([128, 2], F32, tag="sxp")
    nc.vector.tensor_scalar(sxp[:], am[:], EPS, 1.0 / Q8,
                            op0=OP.max, op1=OP.mult)
    nc.sync.dma_start(g["sx_in"].rearrange("(j p) -> p j", p=128), sxp[:])
    nc.gpsimd.collective_compute("AllGather", OP.bypass,
                                 ins=[g["sx_in"][:]], outs=[g["sx_out"][:]],
                                 replica_groups=rg)
    p0_cm.__exit__(None, None, None)

    sx_rep = wp.tile([128, T], F32, tag="sxrep")
    nc.sync.dma_start(sx_rep[:],
                      g["sx_out"].rearrange("(a t) -> a t", a=1)
                      .to_broadcast([128, T]))
    if DEBUG:
        nc.sync.dma_start(g["dbg_sx"][:], sx_rep[0:1, :])
    sx_pp = wp.tile([128, NT], F32, tag="sxpp")
    nc.scalar.dma_start(sx_pp[:], g["sx_out"].rearrange("(j p) -> p j", p=128))

    # -------- P1a: load + quantize x^T -> int bf16 --------
    xq = wp.tile([128, NE * T], BF16, tag="xq")
    p1a_cm, p1a = pool("p1a", bufs=3)
    rsx_rep = p1a.tile([128, T], F32, tag="rsxrep", bufs=1)
    nc.vector.reciprocal(rsx_rep[:], sx_rep[:])
    dma_engs = [nc.sync, nc.scalar, nc.gpsimd]
    rndc_pp = wp.tile([128, 1], F32, tag="rndcpp")
    nc.vector.memset(rndc_pp[:], RND_C)
    for et in range(NE):
        xt = p1a.tile([128, T], F32, tag="xt", bufs=3)
        dma_engs[et % 3].dma_start(xt[:], g["xT_d"][et * 128:(et + 1) * 128, :])
        nc.vector.tensor_tensor(xt[:], xt[:], rsx_rep[:], op=OP.mult)
        nc.vector.tensor_scalar(xt[:], xt[:], RND_C, None, op0=OP.add)
        nc.vector.tensor_scalar(xq[:, et * T:(et + 1) * T], xt[:],
                                RND_C, None, op0=OP.subtract)
    p1a_cm.__exit__(None, None, None)

    # -------- P1b: Q/K/V projections, three e-passes, streamed weights ----
    p1b_cm, p1b = pool("p1b", bufs=3)
    ps1_cm, ps1 = pool("ps1", space="PSUM")

    # Q pass: out^T [o, t] = wq_int^T.T @ xq  (+ scales/bias after)
    for name, wd, sw_pp, b_pp, outs, extra_scale in (
            ("q", g["wq_d"], swq_pp, qb_pp, qT, SCALING),
            ("k", g["wk_d"], swk_pp, kb_pp, kTx, None)):
        pP = [ps1.tile([128, T], F32, tag=f"pP{o}", name=f"pP{o}_{name}")
              for o in range(2)]
        for et in range(NE):
            we = p1b.tile([128, CH], BF16, tag="wstream")
            nc.sync.dma_start(we[:], wd[et * 128:(et + 1) * 128, :])
            xq_e = xq[:, et * T:(et + 1) * T]
            for o in range(2):
                for n in range(4):
                    nsl = slice(n * 512, (n + 1) * 512)
                    nc.tensor.matmul(pP[o][:, nsl],
                                     we[:, o * 128:(o + 1) * 128],
                                     xq_e[:, nsl],
                                     start=(et == 0), stop=(et == NE - 1))
        for o in range(2):
            e1 = p1b.tile([128, T], F32, tag="ev1", bufs=2)
            nc.vector.tensor_scalar(e1[:], pP[o][:], sw_pp[:, o:o + 1],
                                    None, op0=OP.mult)
            nc.vector.tensor_tensor(e1[:], e1[:], sx_rep[:], op=OP.mult)
            if extra_scale is not None:
                nc.vector.tensor_scalar(outs[o][:], e1[:], b_pp[:, o:o + 1],
                                        extra_scale, op0=OP.add, op1=OP.mult)
            else:
                nc.vector.tensor_scalar(outs[o][:], e1[:], b_pp[:, o:o + 1],
                                        None, op0=OP.add)
                nc.vector.tensor_copy(kTr[o][:], outs[o][:])

    ps1_cm.__exit__(None, None, None)
    # V pass: [t, o] = xq_block.T @ wv_int; one accumulation group per bank
    ps1v_cm, ps1v = pool("ps1v", bufs=2, space="PSUM")
    wv_res = p1b.tile([128, NE * CH], BF16, tag="wvres", bufs=1)
    for et in range(NE):
        nc.sync.dma_start(wv_res[:, et * CH:(et + 1) * CH],
                          g["wv_d"][et * 128:(et + 1) * 128, :])
    for j in range(NT):
        pVj = ps1v.tile([128, CH], F32, tag="pV", name=f"pV{j}")
        for et in range(NE):
            xq_e = xq[:, et * T:(et + 1) * T]
            nc.tensor.matmul(pVj[:],
                             xq_e[:, j * 128:(j + 1) * 128],
                             wv_res[:, et * CH:(et + 1) * CH],
                             start=(et == 0), stop=(et == NE - 1))
        ysl = slice(j * CH, (j + 1) * CH)
        e3 = p1b.tile([128, CH], F32, tag="ev3")
        nc.vector.tensor_scalar(e3[:], pVj[:], sx_pp[:, j:j + 1], None,
                                op0=OP.mult)
        nc.vector.tensor_tensor(e3[:], e3[:], swv_rep[:], op=OP.mult)
        nc.vector.tensor_tensor(yv[:, ysl], e3[:], vb_rep[:], op=OP.add)
    ps1v_cm.__exit__(None, None, None)
    p1b_cm.__exit__(None, None, None)
    w_cm.__exit__(None, None, None)

    if DEBUG:
        nc.sync.dma_start(g["dbg_qT"][:], qT[0][:].bitcast(F32))
        nc.sync.dma_start(g["dbg_kT"][:], kTx[0][:])
        nc.sync.dma_start(g["dbg_yv"][:], yv[:, 0:CH])

    # -------- P2/P3: k row-max + v out-quant row-max, AllReduce-max --------
    p23_cm, p23 = pool("p23", bufs=2)
    ps23_cm, ps23 = pool("ps23", bufs=2, space="PSUM")
    kmax_pp = p23.tile([128, NT], F32, tag="kmaxpp", bufs=1)
    for b in range(NT):
        for h in range(HL):
            psT = ps23.tile([128, 128], F32, tag="pstat", name=f"psk{b}_{h}")
            nc.tensor.transpose(psT[:], kTx[h][:, b * 128:(b + 1) * 128],
                                ident[:].bitcast(F32))
            if h == 0:
                nc.vector.tensor_reduce(kmax_pp[:, b:b + 1], psT[:],
                                        axis=AX.X, op=OP.max,
                                        apply_absolute_value=True)
            else:
                tmp_r = p23.tile([128, 1], F32, tag="tmpr")
                nc.vector.tensor_reduce(tmp_r[:], psT[:], axis=AX.X,
                                        op=OP.max, apply_absolute_value=True)
                nc.vector.tensor_tensor(kmax_pp[:, b:b + 1],
                                        kmax_pp[:, b:b + 1], tmp_r[:],
                                        op=OP.max)
    nc.sync.dma_start(g["kv_in"][0, :].rearrange("(j p) -> p j", p=128),
                      kmax_pp[:])

    ym = p23.tile([128, NT], F32, tag="ym")
    for j in range(NT):
        nc.vector.tensor_reduce(ym[:, j:j + 1], yv[:, j * CH:(j + 1) * CH],
                                axis=AX.X, op=OP.max,
                                apply_absolute_value=True)
    nc.sync.dma_start(g["kv_in"][1, :].rearrange("(j p) -> p j", p=128),
                      ym[:])
    nc.gpsimd.collective_compute("AllReduce", OP.max,
                                 ins=[g["kv_in"][:]], outs=[g["kv_out"][:]],
                                 replica_groups=rg)
    ps23_cm.__exit__(None, None, None)
    p23_cm.__exit__(None, None, None)

    def chunks_of(w):
        out, s = [], 0
        while s < w:
            out.append((s, min(512, w - s)))
            s += 512
        return out

    # -------- P5: pass 1 -> accumulated attention score --------
    p5_cm, p5 = pool("p5", bufs=3)
    p5m_cm, p5m = pool("p5m", bufs=2)
    psA_cm, psA_p = pool("psA", space="PSUM")
    psS_cm, psS_p = pool("psS", bufs=4, space="PSUM")
    pA = [psA_p.tile([1, 512], F32, tag=f"pA{n}", name=f"pA{n}") for n in range(4)]
    first_wr = [True] * 4
    for h in range(HL):
        for i in range(NT):
            c_cols = (i + 1) * 128 if causal else T
            nch = _cdiv(c_cols, 512)
            diag_n, diag_off = (i * 128) // 512, (i * 128) % 512
            if not causal:
                mrow = p5m.tile([128, T], F32, tag="mrow")
                nc.sync.dma_start(mrow[:], g["mask_d"][i * 128:(i + 1) * 128, :])
            zz = p5.tile([128, 4], F32, tag="zz")
            pp = []
            for n in range(nch):
                w = min(512, c_cols - n * 512)
                psS = psS_p.tile([128, 512], F32, tag="pS",
                                 name=f"pS_{h}_{i}_{n}")
                nc.tensor.matmul(psS[:, :w],
                                 qT[h][:, i * 128:(i + 1) * 128],
                                 kTr[h][:, n * 512:n * 512 + w],
                                 start=True, stop=True)
                if causal and n == diag_n:
                    nc.vector.tensor_tensor(psS[:, diag_off:diag_off + 128],
                                            psS[:, diag_off:diag_off + 128],
                                            tblk[:], op=OP.add)
                elif not causal:
                    nc.vector.tensor_tensor(psS[:, :w], psS[:, :w],
                                            mrow[:, n * 512:n * 512 + w],
                                            op=OP.add)
                p1t = p5.tile([128, 512], F32R, tag="p1t", bufs=6,
                              name=f"p1t_{h}_{i}_{n}")
                nc.scalar.activation(p1t[:, :w], psS[:, :w], ACTF.Exp,
                                     bias=0.0, scale=1.0,
                                     accum_out=zz[:, n:n + 1])
                if w < 512:
                    nc.vector.memset(p1t[:, w:].bitcast(F32), 0.0)
                pp.append(p1t)
            z = p5.tile([128, 1], F32, tag="z")
            if nch == 1:
                nc.vector.tensor_copy(z[:], zz[:, 0:1])
            else:
                nc.vector.tensor_reduce(z[:], zz[:, :nch], axis=AX.X,
                                        op=OP.add)
            rz = p5.tile([128, 1], F32R, tag="rz")
            with nc.allow_low_precision(reason="fp32r matmul feed"):
                nc.vector.reciprocal(rz[:], z[:])
            for n in range(nch):
                nc.tensor.matmul(pA[n][:], rz[:], pp[n][:],
                                 start=first_wr[n],
                                 stop=(h == HL - 1 and i == NT - 1))
                first_wr[n] = False
    accs = p5.tile([1, T], F32, tag="accs")
    for n in range(4):
        nc.vector.tensor_copy(accs[:, n * 512:(n + 1) * 512], pA[n][:])
    nc.sync.dma_start(g["acc_in"].rearrange("(a t) -> a t", a=1), accs[:])
    nc.gpsimd.collective_compute("AllReduce", OP.add,
                                 ins=[g["acc_in"][:]], outs=[g["acc_out"][:]],
                                 replica_groups=rg)
    psS_cm.__exit__(None, None, None)
    psA_cm.__exit__(None, None, None)
    p5m_cm.__exit__(None, None, None)
    p5_cm.__exit__(None, None, None)

    # -------- P4: v quantization --------
    sv_pp = per.tile([128, HL], F32, tag="svpp")
    p4_cm, p4 = pool("p4", bufs=2)
    vq1 = p4.tile([128, NT * CH], F32, tag="vq1")
    ymf = p4.tile([128, NT], F32, tag="ymf")
    nc.sync.dma_start(ymf[:], g["kv_out"][1, :].rearrange("(j p) -> p j",
                                                          p=128))
    sy = p4.tile([128, NT], F32, tag="sy")
    nc.vector.tensor_scalar(sy[:], ymf[:], EPS, 1.0 / Q8,
                            op0=OP.max, op1=OP.mult)
    rsy = p4.tile([128, NT], F32, tag="rsy")
    nc.vector.reciprocal(rsy[:], sy[:])
    for j in range(NT):
        ysl = slice(j * CH, (j + 1) * CH)
        d1 = p4.tile([128, CH], F32, tag="d1")
        nc.vector.tensor_scalar(d1[:], yv[:, ysl], rsy[:, j:j + 1], RND_C,
                                op0=OP.mult, op1=OP.add)
        nc.vector.tensor_scalar(vq1[:, ysl], d1[:], RND_C, sy[:, j:j + 1],
                                op0=OP.subtract, op1=OP.mult)
    ps4_cm, ps4 = pool("ps4", bufs=2, space="PSUM")
    vmax_pp = p4.tile([128, HL], F32, tag="vmaxpp", bufs=1)
    for j in range(NT):
        for ob in range(HL):
            psT = ps4.tile([128, 128], F32, tag="pstat4", name=f"psv{j}_{ob}")
            nc.tensor.transpose(
                psT[:], vq1[:, j * CH + ob * 128:j * CH + (ob + 1) * 128],
                ident[:].bitcast(F32))
            if j == 0:
                nc.vector.tensor_reduce(vmax_pp[:, ob:ob + 1], psT[:],
                                        axis=AX.X, op=OP.max,
                                        apply_absolute_value=True)
            else:
                tmp_r4 = p4.tile([128, 1], F32, tag="tmpr4")
                nc.vector.tensor_reduce(tmp_r4[:], psT[:], axis=AX.X,
                                        op=OP.max, apply_absolute_value=True)
                nc.vector.tensor_tensor(vmax_pp[:, ob:ob + 1],
                                        vmax_pp[:, ob:ob + 1], tmp_r4[:],
                                        op=OP.max)
    ps4_cm.__exit__(None, None, None)
    nc.vector.tensor_scalar(sv_pp[:], vmax_pp[:], EPS, 1.0 / Q8,
                            op0=OP.max, op1=OP.mult)
    nc.sync.dma_start(g["sv_b"].rearrange("(h p) -> p h", p=128), sv_pp[:])
    sv_row = p4.tile([1, CH], F32, tag="svrow")
    nc.sync.dma_start(sv_row[:], g["sv_b"].rearrange("(a c) -> a c", a=1))
    rsv_row = p4.tile([1, CH], F32, tag="rsvrow")
    nc.vector.reciprocal(rsv_row[:], sv_row[:])
    rsv_rep = p4.tile([128, CH], F32, tag="rsvrep")
    nc.gpsimd.partition_broadcast(rsv_rep[:], rsv_row[:])
    for j in range(NT):
        ysl = slice(j * CH, (j + 1) * CH)
        m1 = p4.tile([128, CH], F32, tag="m1")
        nc.vector.tensor_tensor(m1[:], vq1[:, ysl], rsv_rep[:], op=OP.mult)
        nc.vector.tensor_scalar(m1[:], m1[:], RND_C, None, op0=OP.add)
        nc.vector.tensor_scalar(vqi[:, ysl], m1[:], RND_C, None,
                                op0=OP.subtract)
    p4_cm.__exit__(None, None, None)
    yv_cm.__exit__(None, None, None)


    # -------- P6: top-k flags + k scale selection + k2 ints --------
    sel_cm, selp = pool("selp")
    ssel_rep = selp.tile([128, T], F32, tag="sselrep")
    p6_cm, p6 = pool("p6")
    acc4 = p6.tile([4, 512], F32, tag="acc4")
    nc.sync.dma_start(acc4[:], g["acc_out"].rearrange("(r s) -> r s", r=4))
    if DEBUG:
        nc.sync.dma_start(g["dbg_acc"].rearrange("a (r s) -> (a r) s", r=4),
                          acc4[:])
    rvr = p6.tile([4, 512], F32, tag="rvr")
    nc.sync.dma_start(rvr[:], g["rvr_d"][:])
    nc.vector.tensor_tensor(acc4[:], acc4[:], rvr[:], op=OP.mult)
    nc.vector.tensor_scalar(acc4[:], acc4[:], 1.0 / H, None, op0=OP.mult)
    tkw = p6.tile([4, 512], F32, tag="tkw")
    ton = acc4[:]
    for k_on in range(0, K_TOP, 8):
        k_this = min(k_on + 8, K_TOP) - k_on
        mx8 = p6.tile([4, 8], F32, tag="mx8")
        nc.vector.max(out=mx8[:], in_=ton)
        if k_this < 8:
            nc.vector.memset(mx8[:, k_this:], 0)
        nc.vector.match_replace(out=tkw[:], in_to_replace=mx8[:],
                                in_values=ton, imm_value=0)
        ton = tkw[:]
    nc.vector.tensor_sub(out=tkw[:], in0=acc4[:], in1=tkw[:])
    nc.vector.tensor_scalar(tkw[:], tkw[:], 0.0, None, op0=OP.is_gt)
    nc.sync.dma_start(g["flg_b"].rearrange("(r s) -> r s", r=4), tkw[:])
    flags = p6.tile([1, T], F32, tag="flags")
    nc.sync.dma_start(flags[:], g["flg_b"].rearrange("(a t) -> a t", a=1))
    if DEBUG:
        nc.sync.dma_start(g["dbg_flags"][:], flags[:])
    flags_i = p6.tile([1, T], mybir.dt.int32, tag="flagsi")
    nc.vector.tensor_scalar(flags_i[:], flags[:], 0.0, None, op0=OP.is_gt)

    kmaxf = p6.tile([1, T], F32, tag="kmaxf")
    nc.sync.dma_start(kmaxf[:], g["kv_out"][0:1, :])
    s8 = p6.tile([1, T], F32, tag="s8")
    nc.vector.tensor_scalar(s8[:], kmaxf[:], 1.0 / Q8, EPS,
                            op0=OP.mult, op1=OP.max)
    ssel = p6.tile([1, T], F32, tag="ssel")
    nc.vector.tensor_scalar(ssel[:], kmaxf[:], EPS, 1.0 / Q4,
                            op0=OP.max, op1=OP.mult)
    nc.vector.copy_predicated(ssel[:], flags_i[:], s8[:])
    rssel = p6.tile([1, T], F32, tag="rssel")
    nc.vector.reciprocal(rssel[:], ssel[:])
    rssel_rep = p6.tile([128, T], F32, tag="rsselrep")
    nc.sync.dma_start(g["sselr_b"].rearrange("(a t) -> a t", a=1), ssel[:])
    nc.sync.dma_start(g["rsselr_b"].rearrange("(a t) -> a t", a=1), rssel[:])
    nc.sync.dma_start(ssel_rep[:],
                      g["sselr_b"].rearrange("(a t) -> a t", a=1)
                      .to_broadcast([128, T]))
    nc.sync.dma_start(rssel_rep[:],
                      g["rsselr_b"].rearrange("(a t) -> a t", a=1)
                      .to_broadcast([128, T]))
    if DEBUG:
        nc.sync.dma_start(g["dbg_ssel"][:], ssel[:])
    for h in range(HL):
        kq = p6.tile([128, T], F32, tag="kq")
        nc.vector.tensor_tensor(kq[:], kTx[h][:], rssel_rep[:], op=OP.mult)
        nc.vector.tensor_scalar(kq[:], kq[:], RND_C, None, op0=OP.add)
        nc.vector.tensor_scalar(kq[:], kq[:], RND_C, None, op0=OP.subtract)
        nc.vector.tensor_tensor(kTr[h][:], kq[:], ssel_rep[:], op=OP.mult)
    p6_cm.__exit__(None, None, None)

    if DEBUG:
        nc.sync.dma_start(g["dbg_k2"][:], kTr[0][:].bitcast(F32))
        nc.sync.dma_start(g["dbg_vqi"][:], vqi[:, 0:CH].bitcast(F32))

    # -------- P7: pass 2 attention --------
    ctx_cm, ctxp = pool("ctxp")
    ctxT = [ctxp.tile([128, T], F32, tag=f"ctxT{h}", name=f"ctxT{h}") for h in range(HL)]
    psb_cm, psbp = pool("psbp")
    psb = psbp.tile([128, NT * 512], F32R, tag="psb")
    p7_cm, p7 = pool("p7", bufs=3)
    cmax_pp = p7.tile([128, NT], F32, tag="cmaxpp", bufs=1)
    p7m_cm, p7m = pool("p7m", bufs=2)
    ps7s_cm, ps7S = pool("ps7s", bufs=4, space="PSUM")
    ps7t_cm, ps7T = pool("ps7t", bufs=2, space="PSUM")
    ps7c_cm, ps7C = pool("ps7c", bufs=2, space="PSUM")
    for h in range(HL):
        for i in range(NT):
            c_cols = (i + 1) * 128 if causal else T
            nch = _cdiv(c_cols, 512)
            diag_n, diag_off = (i * 128) // 512, (i * 128) % 512
            if not causal:
                mrow = p7m.tile([128, T], F32, tag="mrow2")
                nc.sync.dma_start(mrow[:], g["mask_d"][i * 128:(i + 1) * 128, :])
            zz = p7.tile([128, 4], F32, tag="zz2")
            pp = []
            for n in range(nch):
                w = min(512, c_cols - n * 512)
                psS = ps7S.tile([128, 512], F32, tag="pS2",
                                name=f"pS2_{h}_{i}_{n}")
                nc.tensor.matmul(psS[:, :w],
                                 qT[h][:, i * 128:(i + 1) * 128],
                                 kTr[h][:, n * 512:n * 512 + w],
                                 start=True, stop=True)
                if causal and n == diag_n:
                    nc.vector.tensor_tensor(psS[:, diag_off:diag_off + 128],
                                            psS[:, diag_off:diag_off + 128],
                                            tblk[:], op=OP.add)
                elif not causal:
                    nc.vector.tensor_tensor(psS[:, :w], psS[:, :w],
                                            mrow[:, n * 512:n * 512 + w],
                                            op=OP.add)
                p2t = p7.tile([128, 512], F32, tag="p2t", bufs=6,
                              name=f"p2t_{h}_{i}_{n}")
                nc.scalar.activation(p2t[:, :w], psS[:, :w], ACTF.Exp,
                                     bias=0.0, scale=1.0,
                                     accum_out=zz[:, n:n + 1])
                pp.append(p2t)
            z = p7.tile([128, 1], F32, tag="z2")
            if nch == 1:
                nc.vector.tensor_copy(z[:], zz[:, 0:1])
            else:
                nc.vector.tensor_reduce(z[:], zz[:, :nch], axis=AX.X,
                                        op=OP.add)
            rz = p7.tile([128, 1], F32, tag="rz2")
            nc.vector.reciprocal(rz[:], z[:])
            pn = []
            for n in range(nch):
                w = min(512, c_cols - n * 512)
                p2n = p7.tile([128, 512], F32R, tag="p2n", bufs=4,
                              name=f"p2n_{h}_{i}_{n}")
                nc.vector.tensor_scalar(p2n[:, :w], pp[n][:, :w], rz[:],
                                        None, op0=OP.mult)
                pn.append(p2n)
            # psb layout: i-block-major — block ib at ib*(NT*128), s-tile j
            # at col j*128 within it. Quad transposes -> ONE 512-wide copy.
            nj = (i + 1) if causal else NT
            ib = i % 4
            for jq in range(0, nj, 4):
                jn = min(4, nj - jq)
                psT = ps7T.tile([128, 512], F32, tag="pT")
                for jj in range(jn):
                    j = jq + jj
                    cn = (j * 128) // 512
                    off = j * 128 - cn * 512
                    nc.tensor.transpose(
                        psT[:, jj * 128:(jj + 1) * 128].bitcast(F32R),
                        pn[cn][:, off:off + 128], ident[:])
                dst = psb[:, ib * (NT * 128) + jq * 128:
                          ib * (NT * 128) + (jq + jn) * 128]
                if (jq // 4) % 2 == 0:
                    nc.scalar.copy(dst, psT[:, :jn * 128])
                else:
                    nc.vector.tensor_copy(dst, psT[:, :jn * 128])
            if i % 4 == 3:
                sb = i // 4
                jmax = (4 * sb + 4) if causal else NT
                if causal:
                    for j in range(4 * sb + 1, jmax):
                        for ibg in range(j - 4 * sb):
                            nc.vector.memset(
                                psb[:, ibg * (NT * 128) + j * 128:
                                    ibg * (NT * 128) +
                                    (j + 1) * 128].bitcast(F32), 0.0)
                psb3 = psb[:].rearrange("p (ib jt) -> p ib jt", ib=4)
                psC = ps7C.tile([128, 512], F32, tag="pC")
                for j in range(jmax):
                    nc.tensor.matmul(
                        psC[:],
                        vqi[:, j * CH + h * 128:j * CH + (h + 1) * 128],
                        psb3[:, :, j * 128:(j + 1) * 128],
                        start=(j == 0), stop=(j == jmax - 1))
                nc.vector.tensor_scalar(ctxT[h][:, sb * 512:(sb + 1) * 512],
                                        psC[:], sv_pp[:, h:h + 1], None,
                                        op0=OP.mult)
        # head complete: ctx absmax partial + shard DMA + per-head AllGather
        agi = g["ag1_in"] if h == 0 else g["ag2_in"]
        nc.sync.dma_start(
            agi[:].rearrange("(p t) -> p t", p=128), ctxT[h][:])
        for bq in range(0, NT, 4):
            psT = ps7T.tile([128, 512], F32, tag="pT", name=f"pTs{h}_{bq}")
            for bb in range(4):
                b = bq + bb
                nc.tensor.transpose(psT[:, bb * 128:(bb + 1) * 128],
                                    ctxT[h][:, b * 128:(b + 1) * 128]
                                    .bitcast(F32),
                                    ident[:].bitcast(F32))
            for bb in range(4):
                b = bq + bb
                if h == 0:
                    nc.vector.tensor_reduce(
                        cmax_pp[:, b:b + 1], psT[:, bb * 128:(bb + 1) * 128],
                        axis=AX.X, op=OP.max, apply_absolute_value=True)
                else:
                    tmp_r8 = p7.tile([128, 1], F32, tag="tmpr8")
                    nc.vector.tensor_reduce(
                        tmp_r8[:], psT[:, bb * 128:(bb + 1) * 128],
                        axis=AX.X, op=OP.max, apply_absolute_value=True)
                    nc.vector.tensor_tensor(cmax_pp[:, b:b + 1],
                                            cmax_pp[:, b:b + 1], tmp_r8[:],
                                            op=OP.max)
        if h == 0:
            nc.gpsimd.collective_compute(
                "AllGather", OP.bypass, ins=[g["ag1_in"][:]],
                outs=[g["ag1_out"][:]], replica_groups=rg)
        else:
            nc.sync.dma_start(
                g["cmx_in"].rearrange("(j p) -> p j", p=128), cmax_pp[:])
            ar_i = nc.gpsimd.collective_compute(
                "AllReduce", OP.max, ins=[g["cmx_in"][:]],
                outs=[g["cmx_out"][:]], replica_groups=rg)
            ag2_i = nc.gpsimd.collective_compute(
                "AllGather", OP.bypass, ins=[g["ag2_in"][:]],
                outs=[g["ag2_out"][:]], replica_groups=rg)
            add_dep_helper(ag2_i.ins, ar_i.ins, True, "cmax AR before ctx AG2")
    ps7c_cm.__exit__(None, None, None)
    ps7t_cm.__exit__(None, None, None)
    ps7s_cm.__exit__(None, None, None)
    p7m_cm.__exit__(None, None, None)
    p7_cm.__exit__(None, None, None)
    psb_cm.__exit__(None, None, None)

    if DEBUG:
        nc.sync.dma_start(g["dbg_ctxT"][:], ctxT[0][:])

    ctx_cm.__exit__(None, None, None)
    sel_cm.__exit__(None, None, None)
    vq_cm.__exit__(None, None, None)
    kx_cm.__exit__(None, None, None)

    # -------- P9: output projection --------
    p9p_cm, p9p = pool("p9p")
    p9_cm, p9 = pool("p9", bufs=2)
    cmax = p9p.tile([1, T], F32, tag="cmax")
    nc.sync.dma_start(cmax[:], g["cmx_out"].rearrange("(a t) -> a t", a=1))
    sc = p9p.tile([1, T], F32, tag="sc")
    nc.vector.tensor_scalar(sc[:], cmax[:], EPS, 1.0 / Q8,
                            op0=OP.max, op1=OP.mult)
    nc.sync.dma_start(g["sc_b"].rearrange("(a t) -> a t", a=1), sc[:])
    sc_rep = p9p.tile([128, T], F32, tag="screp")
    rsc_rep = p9p.tile([128, T], F32, tag="rscrep")
    nc.sync.dma_start(sc_rep[:],
                      g["sc_b"].rearrange("(a t) -> a t", a=1)
                      .to_broadcast([128, T]))
    nc.vector.reciprocal(rsc_rep[:], sc_rep[:])
    rndc9 = p9p.tile([128, 1], F32, tag="rndc9")
    nc.vector.memset(rndc9[:], RND_C)

    wo_sb = p9p.tile([128, NE * CH], BF16, tag="wo")
    for et in range(NE):
        nc.sync.dma_start(wo_sb[:, et * CH:(et + 1) * CH],
                          g["wo_d"][et * 128:(et + 1) * 128, :])

    ps9_cm, ps9 = pool("ps9", space="PSUM")
    pO = [ps9.tile([128, T], F32, tag=f"pO{o}", name=f"pO{o}") for o in range(2)]
    et_order = [e for e in range(NE) if e % 2 == 0] + \
               [e for e in range(NE) if e % 2 == 1]
    for eti, et in enumerate(et_order):
        c0 = et // 2
        ct = p9.tile([128, T], F32, tag="ct", bufs=3)
        ag_src = g["ag1_out"] if (et % 2 == 0) else g["ag2_out"]
        (nc.sync if eti % 2 == 0 else nc.gpsimd).dma_start(
            ct[:], ag_src[c0, :].rearrange("(p t) -> p t", p=128))
        q1 = p9.tile([128, T], F32, tag="q1", bufs=3)
        nc.vector.tensor_tensor(q1[:], ct[:], rsc_rep[:], op=OP.mult)
        nc.scalar.activation(q1[:], q1[:], ACTF.Identity, bias=rndc9[:])
        co = p9.tile([128, T], BF16, tag="co", bufs=3)
        nc.vector.tensor_scalar(co[:], q1[:], RND_C, None, op0=OP.subtract)
        for o in range(2):
            wsl = slice(et * CH + o * 128, et * CH + (o + 1) * 128)
            for n in range(4):
                nsl = slice(n * 512, (n + 1) * 512)
                nc.tensor.matmul(pO[o][:, nsl], wo_sb[:, wsl], co[:, nsl],
                                 start=(eti == 0), stop=(eti == NE - 1))
    for o in range(2):
        f1 = p9.tile([128, T], F32, tag="f1", bufs=1)
        nc.vector.tensor_scalar(f1[:], pO[o][:], swo_pp[:, o:o + 1], None,
                                op0=OP.mult)
        nc.vector.tensor_tensor(f1[:], f1[:], sc_rep[:], op=OP.mult)
        outsb = p9.tile([128, T], F32, tag="outsb", bufs=1)
        nc.vector.tensor_scalar(outsb[:], f1[:], ob_pp[:, o:o + 1], None,
                                op0=OP.add)
        nc.sync.dma_start(g["outT_d"][o * 128:(o + 1) * 128, :], outsb[:])
    ps9_cm.__exit__(None, None, None)
    p9_cm.__exit__(None, None, None)
    p9p_cm.__exit__(None, None, None)
    per_cm.__exit__(None, None, None)


# ==================== host side ====================

_CACHE = {}


def _get_nc(causal):
    if causal not in _CACHE:
        _CACHE[causal] = build(causal)
    return _CACHE[causal]


def _quant_w(w):
    amax = np.max(np.abs(w), axis=-1, keepdims=True)
    s = np.maximum(amax, np.float32(EPS)) / np.float32(Q8)
    wi = np.round((w / s).astype(np.float32))
    return wi, s[:, 0].astype(np.float32)


def kernel(hidden_states, attention_mask, q_w, q_b, k_w, k_b, v_w, v_b,
           o_w, o_b, num_heads):
    hidden_states = np.asarray(hidden_states, dtype=np.float32)
    attention_mask = np.asarray(attention_mask, dtype=np.float32)
    assert int(num_heads) == H
    B, T_, E_ = hidden_states.shape
    assert (B, T_, E_) == (1, T, E)

    x = np.ascontiguousarray(hidden_states[0])        # [T, E]
    xT = np.ascontiguousarray(x.T)                    # [E, T]

    causal_ref = np.triu(np.full((T, T), np.float32(NEG), np.float32), k=1)
    mfull = np.ascontiguousarray(attention_mask[0, 0])
    causal = bool(np.array_equal(mfull, causal_ref))

    nc = _get_nc(causal)

    wqi, sq = _quant_w(np.asarray(q_w, np.float32))
    wki, sk = _quant_w(np.asarray(k_w, np.float32))
    wvi, sv = _quant_w(np.asarray(v_w, np.float32))
    woi, so = _quant_w(np.asarray(o_w, np.float32))

    tblk = np.triu(np.full((128, 128), np.float32(NEG), np.float32), k=1)
    ident = np.eye(128, dtype=np.float32)
    rowvec = np.float32(T) - np.arange(T, dtype=np.float32)
    rvr = (np.float32(1.0) / rowvec).reshape(4, 512).astype(np.float32)

    in_maps = []
    for c in range(NCORES):
        ch = slice(c * CH, (c + 1) * CH)
        im = dict(
            xT=xT,
            xrows=np.ascontiguousarray(x[ch, :]),
            wq=np.ascontiguousarray(wqi[ch, :].T).astype(ml_dtypes.bfloat16),
            wk=np.ascontiguousarray(wki[ch, :].T).astype(ml_dtypes.bfloat16),
            wv=np.ascontiguousarray(wvi[ch, :].T).astype(ml_dtypes.bfloat16),
            wo=np.ascontiguousarray(woi[ch, :].T).astype(ml_dtypes.bfloat16),
            swq=np.ascontiguousarray(sq[ch]),
            swk=np.ascontiguousarray(sk[ch]),
            swv=np.ascontiguousarray(sv[ch]),
            swo=np.ascontiguousarray(so[ch]),
            qb=np.ascontiguousarray(np.asarray(q_b, np.float32)[ch]),
            kb=np.ascontiguousarray(np.asarray(k_b, np.float32)[ch]),
            vb=np.ascontiguousarray(np.asarray(v_b, np.float32)[ch]),
            ob=np.ascontiguousarray(np.asarray(o_b, np.float32)[ch]),
            tblk=tblk, ident=ident, rvr=rvr,
        )
        if not causal:
            im["mask"] = mfull
        in_maps.append(im)

    res = run_bass_kernel_spmd(nc, in_maps, list(range(NCORES)))
    kernel.last_results = res.results
    out = np.empty((T, E), dtype=np.float32)
    for c in range(NCORES):
        out[:, c * CH:(c + 1) * CH] = res.results[c]["outT"].T
    return out.reshape(1, T, E)
